# revision 1
# baseline (speedup 1.0000x reference)
"""Trainium2 Bass kernel for nn_LstmEncoder: two 5-layer LSTM stacks.

Architecture (hardcoded from the problem spec):
  x [256, 256, 8] -> stack1 (F=8 -> H=256, 5 layers) -> stack2 (H=256 -> E=128,
  5 layers) -> returns final hidden of last layer, [256, 128].

Sharding: data-parallel over batch, 32 rows per core on 8 cores; weights
replicated.  Per core, the 10 LSTM layers run as a diagonal wavefront
(layer q processes timestep t = s - q at wavefront step s) so the tensor
engine can pack 4 layers' small matmuls into the 4 column groups of the
128x128 PE array (tile_position col-tiling, batch=32 on psum partitions 32j).

Layouts per core:
  - Gates psum per "pass" [128, 1024]: partitions = (layer-slot, batch),
    free = [I(256) | F(256) | O(256) | G(256)] so one sigmoid covers [0:768]
    and one tanh covers [768:1024].  Pass A = stack1 layers 0-3; pass B =
    stack1 layer 4 + stack2 layers paired two-per-row-group.
  - Input, recurrent and bias contributions all accumulate in psum via
    matmul (bias as a K=1 ones-matmul; layer-0 bias rides a ones-feature
    appended to x).
  - h is transposed each step with PE transposes (psum) + DVE copies so the
    next step's matmuls can use hT tiles [K<=128, 32] as cheap stationaries.
"""

import numpy as np

B, T_FULL, F, E = 256, 256, 8, 128
H = 2 * E          # 256
NL = 5
NCORES = 8
BSH = B // NCORES  # 32


def _gate_perm(Hd):
    """PyTorch gate order [i f g o] -> our column order [i f o g]."""
    return np.concatenate([
        np.arange(0, Hd),
        np.arange(Hd, 2 * Hd),
        np.arange(3 * Hd, 4 * Hd),
        np.arange(2 * Hd, 3 * Hd),
    ])


# ---------------- layer table helpers (chain index q = 0..9) ----------------
# q 0..4: stack1 (Hd=256, gates 1024); q 5..9: stack2 (Hd=128, gates 512).

def _is_s1(q):
    return q < 5


def _pass_of(q):
    return "A" if q < 4 else "B"


def _rows(q):
    """Partition row range of layer q within its pass tile."""
    if q < 4:
        return 32 * q, 32 * q + 32
    if q == 4:
        return 0, 32
    p, _slot = divmod(q - 5, 2)
    return 32 * (p + 1), 32 * (p + 2)


def _slot(q):
    return (q - 5) % 2 if q >= 5 else 0


def _ch_cols(q):
    """(c-block col range, h-block col range) within c_all/h_all [128, 512]."""
    if q < 4:
        return (0, 256)
    if q == 4:
        return (256, 512)
    return (256, 384) if _slot(q) == 0 else (384, 512)


def _own_hT(q):
    """List of (buf_key, col) giving this layer's hT stationaries (K-tiles)."""
    if q < 4:
        return [(("A", 0), 32 * q), (("A", 1), 32 * q)]
    if q == 4:
        return [(("B", 0), 0), (("B", 1), 0)]
    p, slot = divmod(q - 5, 2)
    return [(("B", slot), 32 * (p + 1))]


def _in_hT(q):
    if q == 0:
        return None  # x input
    if 1 <= q <= 4:
        return [(("A", 0), 32 * (q - 1)), (("A", 1), 32 * (q - 1))]
    if q == 5:
        return [(("B", 0), 0), (("B", 1), 0)]
    return _own_hT(q - 1)


def build_nc(T, smax=None):
    import concourse.bass as bass
    import concourse.mybir as mybir
    import concourse.tile as tile
    from concourse import bacc
    from contextlib import ExitStack

    fp = mybir.dt.float32
    AF = mybir.ActivationFunctionType
    # Bacc (not bare Bass): its compile() pipeline moves matmul waits to
    # ldweights and splits multi-wait instructions with event semaphores --
    # TRN2 allows at most one sem wait per instruction.
    nc = bacc.Bacc("TRN2", target_bir_lowering=False)

    # ---------------- DRAM declarations ----------------
    xT_d = nc.dram_tensor("xT", [9, T * BSH], fp, kind="ExternalInput")
    id_d = nc.dram_tensor("ident", [128, 128], fp, kind="ExternalInput")
    on_d = nc.dram_tensor("ones", [1, 32], fp, kind="ExternalInput")
    win_d, whh_d, b_d = {}, {}, {}
    for q in range(10):
        G = 1024 if _is_s1(q) else 512
        kt_in = 1 if q == 0 else (2 if (_is_s1(q) or q == 5) else 1)
        kt_hh = 2 if _is_s1(q) else 1
        for k in range(kt_in):
            kp = 9 if q == 0 else 128
            win_d[q, k] = nc.dram_tensor(f"win{q}_{k}", [kp, G], fp,
                                         kind="ExternalInput")
        for k in range(kt_hh):
            whh_d[q, k] = nc.dram_tensor(f"whh{q}_{k}", [128, G], fp,
                                         kind="ExternalInput")
    for q in range(1, 5):
        b_d[q] = nc.dram_tensor(f"bias{q}", [1, 1024], fp, kind="ExternalInput")
    pb_d = {p: nc.dram_tensor(f"pbias{p}", [1, 1024], fp, kind="ExternalInput")
            for p in range(3)}
    out_d = nc.dram_tensor("out", [BSH, E], fp, kind="ExternalOutput")

    with tile.TileContext(nc) as tc, ExitStack() as ctx:
        wpool = ctx.enter_context(tc.tile_pool(name="weights", bufs=1))
        state = ctx.enter_context(tc.tile_pool(name="state", bufs=1))
        hpool = ctx.enter_context(tc.tile_pool(name="hpool", bufs=2))
        hTpool = ctx.enter_context(tc.tile_pool(name="hTpool", bufs=2))
        apool = ctx.enter_context(tc.tile_pool(name="apool", bufs=2))
        gApool = ctx.enter_context(tc.tile_pool(name="gApool", bufs=2,
                                                space="PSUM"))
        gBpool = ctx.enter_context(tc.tile_pool(name="gBpool", bufs=1,
                                                space="PSUM"))
        tppool = ctx.enter_context(tc.tile_pool(name="tppool", bufs=2,
                                                space="PSUM"))

        def load(dram):
            t = wpool.tile(list(dram.shape), fp, name=f"sb_{dram.name}")
            nc.sync.dma_start(t[:], dram[:])
            return t

        xT_sb = load(xT_d)
        id_sb = load(id_d)
        on_sb = load(on_d)
        win_sb = {k: load(v) for k, v in win_d.items()}
        whh_sb = {k: load(v) for k, v in whh_d.items()}
        b_sb = {k: load(v) for k, v in b_d.items()}
        pb_sb = {k: load(v) for k, v in pb_d.items()}

        c_all = state.tile([128, 512], fp, name="c_all")
        nc.gpsimd.memset(c_all[:], 0.0)

        prev_hT = None
        h_tile = None

        for s in range(T + 9 if smax is None else smax + 1):
            act = [q for q in range(10) if 0 <= s - q <= T - 1]
            actA = [q for q in act if _pass_of(q) == "A"]
            actB = [q for q in act if _pass_of(q) == "B"]

            # a layer's c region may have been scribbled on by earlier
            # pass-wide updates (pair slots share DVE instructions); zero it
            # at the layer's first timestep.
            for q in act:
                if s - q == 0:
                    r0, r1 = _rows(q)
                    cc0, cc1 = _ch_cols(q)
                    nc.gpsimd.memset(c_all[r0:r1, cc0:cc1], 0.0)

            g_tiles = {}
            if actA:
                g_tiles["A"] = gApool.tile([128, 1024], fp, name="gA", tag="gA")
            if actB:
                g_tiles["B"] = gBpool.tile([128, 1024], fp, name="gB", tag="gB")

            # ---------- matmuls, grouped by PE column group ----------
            groups = {j: [] for j in range(4)}

            def colgroup(q):
                if q < 4:
                    return q
                if q == 4:
                    return 0
                return (q - 5) // 2 + 1

            for q in act:
                t = s - q
                g = g_tiles[_pass_of(q)]
                r0, r1 = _rows(q)
                # region -> list of (out, lhsT, rhs, start)
                mms_r = {0: [], 1: []}
                if _is_s1(q):
                    if q >= 1:
                        for r, (n0, n1) in enumerate(((0, 512), (512, 1024))):
                            mms_r[r].append((g[r0:r1, n0:n1], on_sb[0:1, 0:32],
                                             b_sb[q][0:1, n0:n1], True))
                    kt_in = 1 if q == 0 else 2
                    for k in range(kt_in):
                        if q == 0:
                            lh = xT_sb[:, 32 * t:32 * t + 32]
                        else:
                            (bk, col) = _in_hT(q)[k]
                            lh = prev_hT[bk][:, col:col + 32]
                        for r, (n0, n1) in enumerate(((0, 512), (512, 1024))):
                            mms_r[r].append((g[r0:r1, n0:n1], lh,
                                             win_sb[q, k][:, n0:n1],
                                             q == 0 and k == 0))
                    if t > 0:
                        for k in range(2):
                            (bk, col) = _own_hT(q)[k]
                            lh = prev_hT[bk][:, col:col + 32]
                            for r, (n0, n1) in enumerate(((0, 512), (512, 1024))):
                                mms_r[r].append((g[r0:r1, n0:n1], lh,
                                                 whh_sb[q, k][:, n0:n1], False))
                else:
                    # stack2: pair bias emitted by the first active layer of
                    # the pair this wstep.
                    p, slot = divmod(q - 5, 2)
                    first_of_pair = (q == min(x for x in act
                                              if x >= 5 and (x - 5) // 2 == p))
                    if first_of_pair:
                        for r, (n0, n1) in enumerate(((0, 512), (512, 1024))):
                            mms_r[r].append((g[r0:r1, n0:n1], on_sb[0:1, 0:32],
                                             pb_sb[p][0:1, n0:n1], True))
                    kt_in = 2 if q == 5 else 1
                    for k in range(kt_in):
                        (bk, col) = _in_hT(q)[k]
                        lh = prev_hT[bk][:, col:col + 32]
                        for c in range(4):
                            o0 = 256 * c + 128 * slot
                            mms_r[c // 2].append((g[r0:r1, o0:o0 + 128], lh,
                                                  win_sb[q, k][:, 128 * c:128 * c + 128],
                                                  False))
                    if t > 0:
                        (bk, col) = _own_hT(q)[0]
                        lh = prev_hT[bk][:, col:col + 32]
                        for c in range(4):
                            o0 = 256 * c + 128 * slot
                            mms_r[c // 2].append((g[r0:r1, o0:o0 + 128], lh,
                                                  whh_sb[q, 0][:, 128 * c:128 * c + 128],
                                                  False))
                # interleave the two psum-bank regions; mark start/stop
                seq = []
                for i in range(max(len(mms_r[0]), len(mms_r[1]))):
                    for r in range(2):
                        if i < len(mms_r[r]):
                            o, lh, rh, st = mms_r[r][i]
                            last = i == len(mms_r[r]) - 1
                            seq.append((o, lh, rh, st, last))
                groups[colgroup(q)].append((q, seq))

            # round-robin emission across column groups
            gseqs = []
            for j in range(4):
                merged = []
                for _q, seq in groups[j]:
                    merged.extend(seq)
                gseqs.append(merged)
            maxlen = max((len(x) for x in gseqs), default=0)
            for i in range(maxlen):
                for j in range(4):
                    if i < len(gseqs[j]):
                        o, lh, rh, st, sp = gseqs[j][i]
                        nc.tensor.matmul(o, lh, rh, start=st, stop=sp,
                                         skip_group_check=True,
                                         tile_position=(0, 32 * j))

            # ---------- activations + state update, per pass ----------
            h_tile = hpool.tile([128, 512], fp, name="h", tag="h")
            for pas, qs in (("A", actA), ("B", actB)):
                if not qs:
                    continue
                g = g_tiles[pas]
                lo = min(_rows(q)[0] for q in qs)
                hi = max(_rows(q)[1] for q in qs)
                # HW partition-base rule: base 32/96 spans <=32, base 64
                # spans <=64 -> split [32:x>64] ranges at 64.
                segs = [(32, 64), (64, hi)] if (lo == 32 and hi > 64) \
                    else [(lo, hi)]
                c0 = 0 if pas == "A" else 256
                s_ifo = apool.tile([128, 768], fp, name=f"sifo{pas}",
                                   tag=f"sifo{pas}")
                s_g = apool.tile([128, 256], fp, name=f"sg{pas}",
                                 tag=f"sg{pas}")
                tmp1 = apool.tile([128, 256], fp, name=f"tmp1{pas}",
                                  tag=f"tmp1{pas}")
                tmp2 = apool.tile([128, 256], fp, name=f"tmp2{pas}",
                                  tag=f"tmp2{pas}")
                thc = apool.tile([128, 256], fp, name=f"thc{pas}",
                                 tag=f"thc{pas}")
                for lo, hi in segs:
                    nc.scalar.activation(s_ifo[lo:hi, :], g[lo:hi, 0:768],
                                         AF.Sigmoid)
                    nc.scalar.activation(s_g[lo:hi, :], g[lo:hi, 768:1024],
                                         AF.Tanh)
                    nc.vector.tensor_mul(tmp1[lo:hi, :], s_ifo[lo:hi, 256:512],
                                         c_all[lo:hi, c0:c0 + 256])
                    nc.vector.tensor_mul(tmp2[lo:hi, :], s_ifo[lo:hi, 0:256],
                                         s_g[lo:hi, :])
                    nc.vector.tensor_add(c_all[lo:hi, c0:c0 + 256],
                                         tmp1[lo:hi, :], tmp2[lo:hi, :])
                    nc.scalar.activation(thc[lo:hi, :],
                                         c_all[lo:hi, c0:c0 + 256], AF.Tanh)
                    nc.vector.tensor_mul(h_tile[lo:hi, c0:c0 + 256],
                                         s_ifo[lo:hi, 512:768], thc[lo:hi, :])

            # ---------- transposes for next wstep ----------
            cur_hT = {}
            tpt = tppool.tile([128, 512], fp, name="tp", tag="tp")
            for ki, (pas, k) in enumerate((("A", 0), ("A", 1),
                                           ("B", 0), ("B", 1))):
                if (pas == "A" and not actA) or (pas == "B" and not actB):
                    continue
                src = h_tile[:, 128 * ki:128 * ki + 128]
                nc.tensor.transpose(tpt[:, 128 * ki:128 * ki + 128], src,
                                    id_sb[:])
                dst = hTpool.tile([128, 128], fp, name=f"hT{pas}{k}",
                                  tag=f"hT{pas}{k}")
                nc.vector.tensor_copy(dst[:], tpt[:, 128 * ki:128 * ki + 128])
                cur_hT[(pas, k)] = dst
            if prev_hT:
                for bk, v in prev_hT.items():
                    cur_hT.setdefault(bk, v)
            prev_hT = cur_hT

            if s == T + 8:
                nc.sync.dma_start(out_d[:], h_tile[96:128, 256:384])

    nc.finalize()
    return nc


def _prep_weights(inputs):
    """Host-side: transpose/permute all weights into the kernel layouts."""
    p1 = _gate_perm(H)   # 1024
    p2 = _gate_perm(E)   # 512
    w = {}
    w["ident"] = np.eye(128, dtype=np.float32)
    w["ones"] = np.ones((1, 32), np.float32)

    w_ih0_1 = np.asarray(inputs["w_ih0_1"], np.float32)   # [4H, F]
    w_ihr_1 = np.asarray(inputs["w_ihr_1"], np.float32)   # [NL-1, 4H, H]
    w_hh_1 = np.asarray(inputs["w_hh_1"], np.float32)     # [NL, 4H, H]
    b_1 = np.asarray(inputs["b_1"], np.float32)           # [NL, 4H]
    w_ih0_2 = np.asarray(inputs["w_ih0_2"], np.float32)   # [4E, H]
    w_ihr_2 = np.asarray(inputs["w_ihr_2"], np.float32)   # [NL-1, 4E, E]
    w_hh_2 = np.asarray(inputs["w_hh_2"], np.float32)     # [NL, 4E, E]
    b_2 = np.asarray(inputs["b_2"], np.float32)           # [NL, 4E]

    # stack1 layer 0: rows 0-7 = w.T, row 8 = bias (rides x's ones feature)
    w0 = np.empty((9, 1024), np.float32)
    w0[:8] = w_ih0_1.T[:, p1]
    w0[8] = b_1[0][p1]
    w["win0_0"] = np.ascontiguousarray(w0)
    for q in range(5):
        for k in range(2):
            w[f"whh{q}_{k}"] = np.ascontiguousarray(
                w_hh_1[q].T[128 * k:128 * (k + 1), p1])
        if q >= 1:
            for k in range(2):
                w[f"win{q}_{k}"] = np.ascontiguousarray(
                    w_ihr_1[q - 1].T[128 * k:128 * (k + 1), p1])
            w[f"bias{q}"] = np.ascontiguousarray(b_1[q][p1][None, :])
    for q in range(5, 10):
        l2 = q - 5
        if q == 5:
            for k in range(2):
                w[f"win{q}_{k}"] = np.ascontiguousarray(
                    w_ih0_2.T[128 * k:128 * (k + 1), p2])
        else:
            w[f"win{q}_0"] = np.ascontiguousarray(w_ihr_2[l2 - 1].T[:, p2])
        w[f"whh{q}_0"] = np.ascontiguousarray(w_hh_2[l2].T[:, p2])
    for p in range(3):
        pb = np.zeros((1, 1024), np.float32)
        ba = b_2[2 * p][p2]
        for c in range(4):
            pb[0, 256 * c:256 * c + 128] = ba[128 * c:128 * c + 128]
        if 2 * p + 1 < 5:
            bb = b_2[2 * p + 1][p2]
            for c in range(4):
                pb[0, 256 * c + 128:256 * c + 256] = bb[128 * c:128 * c + 128]
        w[f"pbias{p}"] = pb
    return w


def _prep_xt(x_core, T):
    """x shard [32, T, 8] -> [9, T*32] transposed with ones row."""
    xt = np.ones((9, T * BSH), np.float32)
    xt[:8] = np.ascontiguousarray(x_core.transpose(2, 1, 0)).reshape(8, T * BSH)
    return xt


def kernel(**inputs):
    from concourse.bass_utils import run_bass_kernel_spmd

    x = np.asarray(inputs["x"], np.float32).reshape(B, T_FULL, F)
    w = _prep_weights(inputs)

    nc = build_nc(T_FULL)
    in_maps = []
    for c in range(NCORES):
        m = dict(w)
        m["xT"] = _prep_xt(x[BSH * c:BSH * (c + 1)], T_FULL)
        in_maps.append(m)
    res = run_bass_kernel_spmd(nc, in_maps, list(range(NCORES))).results
    out = np.concatenate([np.asarray(r["out"]) for r in res], axis=0)
    return out.astype(np.float32)

